# revision 6
# baseline (speedup 1.0000x reference)
"""Multi-head attention (B=4, S=2048, D=1024, H=16, dk=64) on 8 TRN2 NeuronCores.

Sharding: core c = (batch b = c//2, head-group g = c%2 of 8 heads).
Each core computes its head-group's attention output and the partial output
projection (Wo rows for its heads); the host sums the two partials per batch
and adds the (folded) output bias.

Per-core math, all in fp32r matmuls (fp32 storage, full-rate reduced-precision
multiply; PSUM accumulation fp32):
  QT = (Wq/8)^T X_q^T + bq/8      [512, 2048]  (heads stacked on partitions)
  KT = Wk^T X_k^T + bk            [512, 2048]
  V  = X_v Wv                     [2048, 512]  + ones column per head (aug)
  per head h, Sq-block j:
    S^T_i = K_i Q^T               [128, 512] per Skv tile i (PSUM)
    E_i = exp(S^T_i)              (ScalarE, no max-subtraction needed:
                                   scores ~ N(0,1), max < ~6)
    [U^T; r] += [V_i | 1]^T E_i   [65, 512]  (row 64 = softmax denominators)
    O^T = U^T * (1/r)             (DVE reciprocal + PE outer-product bcast)
  Y^T_partial = Wo_c^T O^T        [1024, 2048]
V-bias and output bias are folded on the host: softmax rows sum to 1, so
bv contributes bv_cat @ Wo + bo to every row.
"""

import numpy as np

B, S, D = 4, 2048, 1024
H, DK = 16, 64
LH = 8                 # heads per core
HK = LH * DK           # 512 (local concat dim)
BLK = 512              # Sq block size
NB = S // BLK          # 4
ST = S // 128          # 16 Skv tiles
KT = D // 128          # 8 contraction tiles over D
MT = HK // 128         # 4 m-tiles over local heads

_CACHE = {}


def _build_program():
    from contextlib import ExitStack
    import concourse.bass as bass
    import concourse.tile as tile
    from concourse import bacc, mybir

    f32 = mybir.dt.float32
    f32r = mybir.dt.float32r
    Exp = mybir.ActivationFunctionType.Exp

    nc = bacc.Bacc("TRN2", target_bir_lowering=False, debug=False, num_devices=8)

    xq_d = nc.dram_tensor("xq_t", [D, S], f32r, kind="ExternalInput")
    xk_d = nc.dram_tensor("xk_t", [D, S], f32r, kind="ExternalInput")
    xv_d = nc.dram_tensor("xv_t", [D, S], f32r, kind="ExternalInput")
    wq_d = nc.dram_tensor("wq", [D, HK], f32r, kind="ExternalInput")
    wk_d = nc.dram_tensor("wk", [D, HK], f32r, kind="ExternalInput")
    wv_d = nc.dram_tensor("wv", [D, HK], f32r, kind="ExternalInput")
    wo_d = nc.dram_tensor("wo", [HK, D], f32r, kind="ExternalInput")
    bq_d = nc.dram_tensor("bq2", [128, MT], f32, kind="ExternalInput")
    bk_d = nc.dram_tensor("bk2", [128, MT], f32, kind="ExternalInput")
    y_d = nc.dram_tensor("y_t", [D, S], f32, kind="ExternalOutput")

    with tile.TileContext(nc) as tc, ExitStack() as ctx:
        wpool = ctx.enter_context(tc.tile_pool(name="w", bufs=2))
        big = ctx.enter_context(tc.tile_pool(name="big", bufs=1))
        xs = ctx.enter_context(tc.tile_pool(name="xs", bufs=3))
        es_pool = ctx.enter_context(tc.tile_pool(name="es", bufs=8))
        ot_pool = ctx.enter_context(tc.tile_pool(name="ot", bufs=2))
        rpool = ctx.enter_context(tc.tile_pool(name="r", bufs=2))
        upool = ctx.enter_context(tc.tile_pool(name="u", bufs=2))
        ypool = ctx.enter_context(tc.tile_pool(name="y", bufs=3))
        psA = ctx.enter_context(tc.tile_pool(name="psA", bufs=4, space="PSUM"))
        psU = ctx.enter_context(tc.tile_pool(name="psU", bufs=3, space="PSUM"))
        psB = ctx.enter_context(tc.tile_pool(name="psB", bufs=1, space="PSUM"))

        bq_sb = big.tile([128, MT], f32)
        bk_sb = big.tile([128, MT], f32)
        nc.sync.dma_start(bq_sb[:], bq_d[:])
        nc.sync.dma_start(bk_sb[:], bk_d[:])
        ones_col = big.tile([1, DK], f32r)
        # DVE Memset can't emit f32r; write the 1.0f bit pattern via u32 view
        nc.vector.memset(ones_col[:].bitcast(mybir.dt.uint32), 0x3F800000)

        qt = big.tile([128, MT, S], f32r)
        kt_ = big.tile([128, MT, S], f32r)
        vaug = big.tile([128, ST, LH, DK + 1], f32r)
        # flat memset (strided 4D memset is invalid ISA); V-proj drains
        # overwrite cols 0:DK per head, leaving the aug ones-column intact
        nc.vector.memset(vaug[:, :, :, :].bitcast(mybir.dt.uint32), 0x3F800000)

        def proj_qk(x_dram, w_dram, bias_sb, dst):
            # dst[p, mt, s] = sum_d w[d, mt*128+p] * x^T[d, s] + bias
            w_sb = wpool.tile([128, KT, HK], f32r, tag="w")
            nc.sync.dma_start(w_sb[:], w_dram.ap().rearrange("(kt p) m -> p kt m", p=128))
            for j in range(NB):
                pss = [psA.tile([128, BLK], f32, tag="psA", name=f"ps_proj{j}_{m}") for m in range(MT)]
                for kt in range(KT):
                    xt = xs.tile([128, BLK], f32r, tag="xs")
                    nc.sync.dma_start(
                        xt[:], x_dram[kt * 128 : (kt + 1) * 128, j * BLK : (j + 1) * BLK]
                    )
                    for mt in range(MT):
                        nc.tensor.matmul(
                            pss[mt][:],
                            w_sb[:, kt, mt * 128 : (mt + 1) * 128],
                            xt[:],
                            start=(kt == 0),
                            stop=(kt == KT - 1),
                            skip_group_check=True,
                        )
                for mt in range(MT):
                    nc.vector.tensor_scalar_add(
                        dst[:, mt, j * BLK : (j + 1) * BLK], pss[mt][:],
                        bias_sb[:, mt : mt + 1],
                    )

        proj_qk(xk_d, wk_d, bk_sb, kt_)
        proj_qk(xq_d, wq_d, bq_sb, qt)

        # V projection: V[st*128+p, h*64+k] per Skv tile st, drained into vaug
        wv_sb = wpool.tile([128, KT, HK], f32r, tag="w")
        nc.sync.dma_start(wv_sb[:], wv_d.ap().rearrange("(kt p) m -> p kt m", p=128))
        for j in range(NB):
            pss = [psA.tile([128, HK], f32, tag="psA", name=f"ps_v{j}_{q2}") for q2 in range(4)]
            for kt in range(KT):
                xt = xs.tile([128, BLK], f32r, tag="xs")
                nc.sync.dma_start(
                    xt[:], xv_d[kt * 128 : (kt + 1) * 128, j * BLK : (j + 1) * BLK]
                )
                for q in range(4):
                    nc.tensor.matmul(
                        pss[q][:],
                        xt[:, q * 128 : (q + 1) * 128],
                        wv_sb[:, kt, :],
                        start=(kt == 0),
                        stop=(kt == KT - 1),
                        skip_group_check=True,
                    )
            for q in range(4):
                st = j * 4 + q
                nc.vector.tensor_copy(
                    vaug[:, st, :, 0:DK],
                    pss[q][:].rearrange("p (h k) -> p h k", h=LH),
                )

        # Attention + output projection, fused per Sq-block
        wo_sb = wpool.tile([128, MT, D], f32r, tag="w")  # same slot bytes as proj weights
        nc.sync.dma_start(
            wo_sb[:], wo_d.ap().rearrange("(kt p) m -> p kt m", p=128)
        )
        for j in range(NB):
            otj = ot_pool.tile([128, MT, BLK], f32r)
            for hp in range(LH // 2):
                mt = hp
                psu = [psU.tile([DK + 1, BLK], f32, tag="psU", name=f"ps_u{j}_{hp}_{p2}") for p2 in range(2)]
                for i in range(ST):
                    for pi in range(2):
                        h = 2 * hp + pi
                        bp = pi * 64
                        pss = psA.tile([128, BLK], f32, tag="psA")
                        nc.tensor.matmul(
                            pss[:],
                            kt_[bp : bp + 64, mt, i * 128 : (i + 1) * 128],
                            qt[bp : bp + 64, mt, j * BLK : (j + 1) * BLK],
                            start=True,
                            stop=True,
                            skip_group_check=True,
                        )
                        es = es_pool.tile([128, BLK], f32r, tag="es")
                        nc.scalar.activation(es[:], pss[:], Exp)
                        nc.tensor.matmul(
                            psu[pi][:],
                            vaug[:, i, h, :],
                            es[:],
                            start=(i == 0),
                            stop=(i == ST - 1),
                            skip_group_check=True,
                        )
                for pi in range(2):
                    h = 2 * hp + pi
                    bp = pi * 64
                    rinv = rpool.tile([1, BLK], f32r, tag="r")
                    with nc.allow_low_precision(reason="f32r rounding of 1/r, ~1e-4 rel"):
                        nc.vector.reciprocal(rinv[:], psu[pi][DK : DK + 1, :])
                    psb = psB.tile([DK, BLK], f32, tag="psB")
                    nc.tensor.matmul(
                        psb[:], ones_col[:], rinv[:],
                        start=True, stop=True, skip_group_check=True,
                    )
                    usb = upool.tile([DK, BLK], f32, tag="u")
                    nc.vector.tensor_copy(usb[:], psu[pi][0:DK, :])
                    nc.vector.tensor_mul(otj[bp : bp + 64, mt, :], usb[:], psb[:])
            for mo in range(KT):
                psy = psA.tile([128, BLK], f32, tag="psA")
                for kt in range(MT):
                    nc.tensor.matmul(
                        psy[:],
                        wo_sb[:, kt, mo * 128 : (mo + 1) * 128],
                        otj[:, kt, :],
                        start=(kt == 0),
                        stop=(kt == MT - 1),
                        skip_group_check=True,
                    )
                ysb = ypool.tile([128, BLK], f32, tag="y")
                nc.vector.tensor_copy(ysb[:], psy[:])
                nc.sync.dma_start(
                    y_d[mo * 128 : (mo + 1) * 128, j * BLK : (j + 1) * BLK], ysb[:]
                )

    nc.compile()
    return nc


def get_program():
    if "nc" not in _CACHE:
        _CACHE["nc"] = _build_program()
    return _CACHE["nc"]


def make_core_inputs(query, key, value, Wq, bq, Wk, bk, Wv, bv, Wo, bo):
    """Build the 8 per-core input dicts (and the folded output bias)."""
    f = np.float32
    in_maps = []
    for c in range(8):
        b, g = c // 2, c % 2
        hs = slice(g * LH, (g + 1) * LH)
        m = {
            "xq_t": np.ascontiguousarray(query[b].T, dtype=f),
            "xk_t": np.ascontiguousarray(key[b].T, dtype=f),
            "xv_t": np.ascontiguousarray(value[b].T, dtype=f),
            "wq": np.ascontiguousarray(
                Wq[hs].transpose(1, 0, 2).reshape(D, HK) / 8.0, dtype=f
            ),
            "wk": np.ascontiguousarray(
                Wk[hs].transpose(1, 0, 2).reshape(D, HK), dtype=f
            ),
            "wv": np.ascontiguousarray(
                Wv[hs].transpose(1, 0, 2).reshape(D, HK), dtype=f
            ),
            "wo": np.ascontiguousarray(Wo[g * HK : (g + 1) * HK, :], dtype=f),
            "bq2": np.ascontiguousarray(
                (bq[hs].reshape(HK) / 8.0).reshape(MT, 128).T, dtype=f
            ),
            "bk2": np.ascontiguousarray(
                bk[hs].reshape(HK).reshape(MT, 128).T, dtype=f
            ),
        }
        in_maps.append(m)
    bo_eff = (bv.reshape(H * DK).astype(np.float64) @ Wo.astype(np.float64)
              + bo.astype(np.float64)).astype(f)
    return in_maps, bo_eff


def combine_outputs(results, bo_eff):
    """results: list of 8 dicts with 'y_t' [D, S]. Returns [B, S, D] f32."""
    out = np.empty((B, S, D), dtype=np.float32)
    for b in range(B):
        acc = results[2 * b]["y_t"] + results[2 * b + 1]["y_t"]
        out[b] = acc.T + bo_eff[None, :]
    return out


def kernel(**inputs):
    from concourse.bass_utils import run_bass_kernel_spmd

    inputs = {k: np.asarray(v) for k, v in inputs.items()}
    nc = get_program()
    in_maps, bo_eff = make_core_inputs(
        inputs["query"], inputs["key"], inputs["value"],
        inputs["Wq"], inputs["bq"], inputs["Wk"], inputs["bk"],
        inputs["Wv"], inputs["bv"], inputs["Wo"], inputs["bo"],
    )
    res = run_bass_kernel_spmd(nc, in_maps, list(range(8)))
    return combine_outputs(res.results, bo_eff)


# revision 8
# speedup vs baseline: 1.7379x; 1.7379x over previous
"""Multi-head attention (B=4, S=2048, D=1024, H=16, dk=64) on 8 TRN2 NeuronCores.

Sharding: core c = (batch b = c//2, head-group g = c%2 of 8 heads).
Each core computes its head-group's attention output and the partial output
projection (Wo rows for its heads); the host sums the two partials per batch
and adds the (folded) output bias.

Per-core math, all in fp32r matmuls (fp32 storage, full-rate reduced-precision
multiply; PSUM accumulation fp32):
  QT = (Wq/8)^T X_q^T + bq/8      [512, 2048]  (heads stacked on partitions)
  KT = Wk^T X_k^T + bk            [512, 2048]
  V  = X_v Wv                     [2048, 512]  + ones column per head (aug)
  per head-pair hp = (h0, h1), Sq-block j:
    S^T_i pair packed in PE row groups 0/64 -> one 2-bank PSUM [128, 1024]
    E_i = exp(S^T pair)           one ScalarE op over 1024 (no max-subtraction
                                   needed: scores ~ N(0,1), max < ~6)
    [U^T; r] += [V_i | 1]^T E_i   [65, 512] per head (row 64 = denominators)
    O^T = U^T * (1/r)             (DVE reciprocal_approx_fast + PE outer bcast)
  Y^T_partial = Wo_c^T O^T        [1024, 2048]
V-bias and output bias are folded on the host: softmax rows sum to 1, so
bv contributes bv_cat @ Wo + bo to every row.

Perf notes (HW-measured):
  - f32r matmul N=512 runs at 227 ns (full rate) incl. overlapped LDWEIGHTS.
  - K=64 matmuls on a single row group run at half rate; alternating row
    groups (base partitions 0/64) makes the pair run concurrently.
  - A [128,1024] 2-bank PSUM tile lets one EXP drain a whole score pair.
"""

import numpy as np

B, S, D = 4, 2048, 1024
H, DK = 16, 64
LH = 8                 # heads per core
HK = LH * DK           # 512 (local concat dim)
BLK = 512              # Sq block size
NB = S // BLK          # 4
ST = S // 128          # 16 Skv tiles
KT = D // 128          # 8 contraction tiles over D
MT = HK // 128         # 4 m-tiles over local heads

_CACHE = {}


def _build_program():
    from contextlib import ExitStack
    import concourse.bass as bass
    import concourse.tile as tile
    from concourse import bacc, mybir

    f32 = mybir.dt.float32
    f32r = mybir.dt.float32r
    u32 = mybir.dt.uint32
    Exp = mybir.ActivationFunctionType.Exp

    nc = bacc.Bacc("TRN2", target_bir_lowering=False, debug=False, num_devices=8)

    xq_d = nc.dram_tensor("xq_t", [D, S], f32r, kind="ExternalInput")
    xk_d = nc.dram_tensor("xk_t", [D, S], f32r, kind="ExternalInput")
    xv_d = nc.dram_tensor("xv_t", [D, S], f32r, kind="ExternalInput")
    wq_d = nc.dram_tensor("wq", [D, HK], f32r, kind="ExternalInput")
    wk_d = nc.dram_tensor("wk", [D, HK], f32r, kind="ExternalInput")
    wv_d = nc.dram_tensor("wv", [D, HK], f32r, kind="ExternalInput")
    wo_d = nc.dram_tensor("wo", [HK, D], f32r, kind="ExternalInput")
    bq_d = nc.dram_tensor("bq2", [128, MT], f32, kind="ExternalInput")
    bk_d = nc.dram_tensor("bk2", [128, MT], f32, kind="ExternalInput")
    y_d = nc.dram_tensor("y_t", [D, S], f32, kind="ExternalOutput")

    with tile.TileContext(nc) as tc, ExitStack() as ctx:
        wpool = ctx.enter_context(tc.tile_pool(name="w", bufs=2))
        big = ctx.enter_context(tc.tile_pool(name="big", bufs=1))
        xs = ctx.enter_context(tc.tile_pool(name="xs", bufs=4))
        es_pool = ctx.enter_context(tc.tile_pool(name="es", bufs=4))
        ot_pool = ctx.enter_context(tc.tile_pool(name="ot", bufs=2))
        rpool = ctx.enter_context(tc.tile_pool(name="r", bufs=3))
        upool = ctx.enter_context(tc.tile_pool(name="u", bufs=3))
        ypool = ctx.enter_context(tc.tile_pool(name="y", bufs=3))
        # PSUM: psS 2x[128,1024] (4 banks) + psU 3x[65,512] + psB 1x[64,512] = 8
        psS = ctx.enter_context(tc.tile_pool(name="psS", bufs=2, space="PSUM"))
        psU = ctx.enter_context(tc.tile_pool(name="psU", bufs=3, space="PSUM"))
        psB = ctx.enter_context(tc.tile_pool(name="psB", bufs=1, space="PSUM"))

        bq_sb = big.tile([128, MT], f32)
        bk_sb = big.tile([128, MT], f32)
        nc.sync.dma_start(bq_sb[:], bq_d[:])
        nc.sync.dma_start(bk_sb[:], bk_d[:])
        ones_col = big.tile([1, DK], f32r)
        # DVE Memset can't emit f32r; write the 1.0f bit pattern via u32 view
        nc.vector.memset(ones_col[:].bitcast(u32), 0x3F800000)

        qt = big.tile([128, MT, S], f32r)
        kt_ = big.tile([128, MT, S], f32r)
        vaug = big.tile([128, ST, LH, DK + 1], f32r)
        # flat memset (strided 4D memset is invalid ISA); V-proj drains
        # overwrite cols 0:DK per head, leaving the aug ones-column intact
        nc.vector.memset(vaug[:, :, :, :].bitcast(u32), 0x3F800000)

        def proj_qk(x_dram, w_dram, bias_sb, dst, wname):
            # dst[p, mt, s] = sum_d w[d, mt*128+p] * x^T[d, s] + bias
            # mt pairs share one 2-bank PSUM tile (left/right halves)
            w_sb = wpool.tile([128, KT, HK], f32r, tag="w", name=f"w_{wname}")
            nc.sync.dma_start(w_sb[:], w_dram.ap().rearrange("(kt p) m -> p kt m", p=128))
            for j in range(NB):
                pp = [psS.tile([128, 2 * BLK], f32, tag="psS", name=f"pp_{wname}{j}_{t}")
                      for t in range(2)]
                for kt in range(KT):
                    xt = xs.tile([128, BLK], f32r, tag="xs", name=f"xt_{wname}{j}_{kt}")
                    nc.sync.dma_start(
                        xt[:], x_dram[kt * 128 : (kt + 1) * 128, j * BLK : (j + 1) * BLK]
                    )
                    for mt in range(MT):
                        half = (mt % 2) * BLK
                        nc.tensor.matmul(
                            pp[mt // 2][:, half : half + BLK],
                            w_sb[:, kt, mt * 128 : (mt + 1) * 128],
                            xt[:],
                            start=(kt == 0),
                            stop=(kt == KT - 1),
                            skip_group_check=True,
                        )
                for mt in range(MT):
                    half = (mt % 2) * BLK
                    nc.vector.tensor_scalar_add(
                        dst[:, mt, j * BLK : (j + 1) * BLK],
                        pp[mt // 2][:, half : half + BLK],
                        bias_sb[:, mt : mt + 1],
                    )

        proj_qk(xk_d, wk_d, bk_sb, kt_, "k")
        proj_qk(xq_d, wq_d, bq_sb, qt, "q")

        # V projection: V[st*128+p, h*64+k] per Skv tile st, drained into vaug
        wv_sb = wpool.tile([128, KT, HK], f32r, tag="w")
        nc.sync.dma_start(wv_sb[:], wv_d.ap().rearrange("(kt p) m -> p kt m", p=128))
        for j in range(NB):
            pp = [psS.tile([128, 2 * BLK], f32, tag="psS", name=f"pp_v{j}_{t}")
                  for t in range(2)]
            for kt in range(KT):
                xt = xs.tile([128, BLK], f32r, tag="xs", name=f"xt_v{j}_{kt}")
                nc.sync.dma_start(
                    xt[:], xv_d[kt * 128 : (kt + 1) * 128, j * BLK : (j + 1) * BLK]
                )
                for q in range(4):
                    half = (q % 2) * BLK
                    nc.tensor.matmul(
                        pp[q // 2][:, half : half + BLK],
                        xt[:, q * 128 : (q + 1) * 128],
                        wv_sb[:, kt, :],
                        start=(kt == 0),
                        stop=(kt == KT - 1),
                        skip_group_check=True,
                    )
            for q in range(4):
                st = j * 4 + q
                half = (q % 2) * BLK
                nc.vector.tensor_copy(
                    vaug[:, st, :, 0:DK],
                    pp[q // 2][:, half : half + BLK].rearrange("p (h k) -> p h k", h=LH),
                )

        # Attention + output projection, fused per Sq-block
        wo_sb = wpool.tile([128, MT, D], f32r, tag="w")  # same slot bytes as proj weights
        nc.sync.dma_start(
            wo_sb[:], wo_d.ap().rearrange("(kt p) m -> p kt m", p=128)
        )
        for j in range(NB):
            otj = ot_pool.tile([128, MT, BLK], f32r)
            for hp in range(LH // 2):
                mt = hp
                psu = [psU.tile([DK + 1, BLK], f32, tag="psU",
                                name=f"ps_u{j}_{hp}_{p2}") for p2 in range(2)]
                for i in range(ST):
                    ps2 = psS.tile([128, 2 * BLK], f32, tag="psS",
                                   name=f"ps_s{j}_{hp}_{i}")
                    for pi in range(2):
                        bp = pi * 64
                        nc.tensor.matmul(
                            ps2[:, pi * BLK : (pi + 1) * BLK],
                            kt_[bp : bp + 64, mt, i * 128 : (i + 1) * 128],
                            qt[bp : bp + 64, mt, j * BLK : (j + 1) * BLK],
                            start=True,
                            stop=True,
                            skip_group_check=True,
                        )
                    es = es_pool.tile([128, 2 * BLK], f32r, tag="es")
                    nc.scalar.activation(es[:], ps2[:], Exp)
                    for pi in range(2):
                        h = 2 * hp + pi
                        nc.tensor.matmul(
                            psu[pi][:],
                            vaug[:, i, h, :],
                            es[:, pi * BLK : (pi + 1) * BLK],
                            start=(i == 0),
                            stop=(i == ST - 1),
                            skip_group_check=True,
                        )
                for pi in range(2):
                    bp = pi * 64
                    # r row (psum partition 64) -> partition 0; PE outer-product
                    # broadcasts r to [64, BLK]; then approx-reciprocal aligned
                    # at partition 0 (the custom DVE op ignores input partition
                    # offsets, so it must read from partition 0).
                    rrow = rpool.tile([1, BLK], f32r, tag="r", name=f"rr{j}_{hp}_{pi}")
                    nc.vector.tensor_copy(rrow[:], psu[pi][DK : DK + 1, :])
                    psb = psB.tile([DK, BLK], f32, tag="psB", name=f"psb{j}_{hp}_{pi}")
                    nc.tensor.matmul(
                        psb[:], ones_col[:], rrow[:],
                        start=True, stop=True, skip_group_check=True,
                    )
                    rbc = upool.tile([DK, BLK], f32, tag="rb", name=f"rb{j}_{hp}_{pi}")
                    nc.vector.reciprocal_approx_fast(rbc[:], psb[:])
                    usb = upool.tile([DK, BLK], f32, tag="u", name=f"usb{j}_{hp}_{pi}")
                    nc.vector.tensor_copy(usb[:], psu[pi][0:DK, :])
                    nc.vector.tensor_mul(otj[bp : bp + 64, mt, :], usb[:], rbc[:])
            # output projection for this Sq-block; mo pairs share a 2-bank tile
            for mp in range(KT // 2):
                psy = psS.tile([128, 2 * BLK], f32, tag="psS", name=f"psy{j}_{mp}")
                for half in range(2):
                    mo = 2 * mp + half
                    for kt in range(MT):
                        nc.tensor.matmul(
                            psy[:, half * BLK : (half + 1) * BLK],
                            wo_sb[:, kt, mo * 128 : (mo + 1) * 128],
                            otj[:, kt, :],
                            start=(kt == 0),
                            stop=(kt == MT - 1),
                            skip_group_check=True,
                        )
                for half in range(2):
                    mo = 2 * mp + half
                    ysb = ypool.tile([128, BLK], f32, tag="y", name=f"ysb{j}_{mp}_{half}")
                    nc.vector.tensor_copy(ysb[:], psy[:, half * BLK : (half + 1) * BLK])
                    nc.sync.dma_start(
                        y_d[mo * 128 : (mo + 1) * 128, j * BLK : (j + 1) * BLK], ysb[:]
                    )

    nc.compile()
    return nc


def get_program():
    if "nc" not in _CACHE:
        _CACHE["nc"] = _build_program()
    return _CACHE["nc"]


def make_core_inputs(query, key, value, Wq, bq, Wk, bk, Wv, bv, Wo, bo):
    """Build the 8 per-core input dicts (and the folded output bias)."""
    f = np.float32
    in_maps = []
    for c in range(8):
        b, g = c // 2, c % 2
        hs = slice(g * LH, (g + 1) * LH)
        m = {
            "xq_t": np.ascontiguousarray(query[b].T, dtype=f),
            "xk_t": np.ascontiguousarray(key[b].T, dtype=f),
            "xv_t": np.ascontiguousarray(value[b].T, dtype=f),
            "wq": np.ascontiguousarray(
                Wq[hs].transpose(1, 0, 2).reshape(D, HK) / 8.0, dtype=f
            ),
            "wk": np.ascontiguousarray(
                Wk[hs].transpose(1, 0, 2).reshape(D, HK), dtype=f
            ),
            "wv": np.ascontiguousarray(
                Wv[hs].transpose(1, 0, 2).reshape(D, HK), dtype=f
            ),
            "wo": np.ascontiguousarray(Wo[g * HK : (g + 1) * HK, :], dtype=f),
            "bq2": np.ascontiguousarray(
                (bq[hs].reshape(HK) / 8.0).reshape(MT, 128).T, dtype=f
            ),
            "bk2": np.ascontiguousarray(
                bk[hs].reshape(HK).reshape(MT, 128).T, dtype=f
            ),
        }
        in_maps.append(m)
    bo_eff = (bv.reshape(H * DK).astype(np.float64) @ Wo.astype(np.float64)
              + bo.astype(np.float64)).astype(f)
    return in_maps, bo_eff


def combine_outputs(results, bo_eff):
    """results: list of 8 dicts with 'y_t' [D, S]. Returns [B, S, D] f32."""
    out = np.empty((B, S, D), dtype=np.float32)
    for b in range(B):
        acc = results[2 * b]["y_t"] + results[2 * b + 1]["y_t"]
        out[b] = acc.T + bo_eff[None, :]
    return out


def kernel(**inputs):
    from concourse.bass_utils import run_bass_kernel_spmd

    inputs = {k: np.asarray(v) for k, v in inputs.items()}
    nc = get_program()
    in_maps, bo_eff = make_core_inputs(
        inputs["query"], inputs["key"], inputs["value"],
        inputs["Wq"], inputs["bq"], inputs["Wk"], inputs["bk"],
        inputs["Wv"], inputs["bv"], inputs["Wo"], inputs["bo"],
    )
    res = run_bass_kernel_spmd(nc, in_maps, list(range(8)))
    return combine_outputs(res.results, bo_eff)


# revision 9
# speedup vs baseline: 1.8636x; 1.0723x over previous
"""Multi-head attention (B=4, S=2048, D=1024, H=16, dk=64) on 8 TRN2 NeuronCores.

Sharding: core c = (batch b = c//2, head-group g = c%2 of 8 heads).
Each core computes its head-group's attention output and the partial output
projection (Wo rows for its heads); the host sums the two partials per batch
and adds the (folded) output bias.

Per-core math, all in fp32r matmuls (fp32 storage, full-rate reduced-precision
multiply; PSUM accumulation fp32):
  QT = (Wq/8)^T X_q^T + bq/8      [512, 2048]  (heads stacked on partitions)
  KT = Wk^T X_k^T + bk            [512, 2048]
  V  = X_v Wv                     [2048, 512]  + ones column per head (aug)
  per head-pair hp = (h0, h1), Sq-block j:
    S^T_i pair packed in PE row groups 0/64 -> one 2-bank PSUM [128, 1024]
    E_i = exp(S^T pair)           one ScalarE op over 1024 (no max-subtraction
                                   needed: scores ~ N(0,1), max < ~6)
    [U^T; r] += [V_i | 1]^T E_i   [65, 512] per head (row 64 = denominators)
    O^T = U^T * (1/r)             (DVE reciprocal_approx_fast + PE outer bcast)
  Y^T_partial = Wo_c^T O^T        [1024, 2048]
V-bias and output bias are folded on the host: softmax rows sum to 1, so
bv contributes bv_cat @ Wo + bo to every row.

Perf notes (HW-measured):
  - f32r matmul N=512 runs at 227 ns (full rate) incl. overlapped LDWEIGHTS.
  - K=64 matmuls on a single row group run at half rate; alternating row
    groups (base partitions 0/64) makes the pair run concurrently.
  - A [128,1024] 2-bank PSUM tile lets one EXP drain a whole score pair.
"""

import numpy as np

B, S, D = 4, 2048, 1024
H, DK = 16, 64
LH = 8                 # heads per core
HK = LH * DK           # 512 (local concat dim)
BLK = 512              # Sq block size
NB = S // BLK          # 4
ST = S // 128          # 16 Skv tiles
KT = D // 128          # 8 contraction tiles over D
MT = HK // 128         # 4 m-tiles over local heads

_CACHE = {}


def _build_program():
    from contextlib import ExitStack
    import concourse.bass as bass
    import concourse.tile as tile
    from concourse import bacc, mybir

    f32 = mybir.dt.float32
    f32r = mybir.dt.float32r
    u32 = mybir.dt.uint32
    Exp = mybir.ActivationFunctionType.Exp

    nc = bacc.Bacc("TRN2", target_bir_lowering=False, debug=False, num_devices=8)

    xq_d = nc.dram_tensor("xq_t", [D, S], f32r, kind="ExternalInput")
    xk_d = nc.dram_tensor("xk_t", [D, S], f32r, kind="ExternalInput")
    xv_d = nc.dram_tensor("xv_t", [D, S], f32r, kind="ExternalInput")
    wq_d = nc.dram_tensor("wq", [D, HK], f32r, kind="ExternalInput")
    wk_d = nc.dram_tensor("wk", [D, HK], f32r, kind="ExternalInput")
    wv_d = nc.dram_tensor("wv", [D, HK], f32r, kind="ExternalInput")
    wo_d = nc.dram_tensor("wo", [HK, D], f32r, kind="ExternalInput")
    bq_d = nc.dram_tensor("bq2", [128, MT], f32, kind="ExternalInput")
    bk_d = nc.dram_tensor("bk2", [128, MT], f32, kind="ExternalInput")
    y_d = nc.dram_tensor("y_t", [D, S], f32, kind="ExternalOutput")

    with tile.TileContext(nc) as tc, ExitStack() as ctx:
        wpool = ctx.enter_context(tc.tile_pool(name="w", bufs=2))
        big = ctx.enter_context(tc.tile_pool(name="big", bufs=1))
        xs = ctx.enter_context(tc.tile_pool(name="xs", bufs=6))
        es_pool = ctx.enter_context(tc.tile_pool(name="es", bufs=6))
        ot_pool = ctx.enter_context(tc.tile_pool(name="ot", bufs=2))
        rpool = ctx.enter_context(tc.tile_pool(name="r", bufs=3))
        upool = ctx.enter_context(tc.tile_pool(name="u", bufs=3))
        ypool = ctx.enter_context(tc.tile_pool(name="y", bufs=3))
        # PSUM: psS 2x[128,1024] (4 banks) + psU 3x[65,512] + psB 1x[64,512] = 8
        psS = ctx.enter_context(tc.tile_pool(name="psS", bufs=2, space="PSUM"))
        psU = ctx.enter_context(tc.tile_pool(name="psU", bufs=3, space="PSUM"))
        psB = ctx.enter_context(tc.tile_pool(name="psB", bufs=1, space="PSUM"))

        bq_sb = big.tile([128, MT], f32)
        bk_sb = big.tile([128, MT], f32)
        nc.sync.dma_start(bq_sb[:], bq_d[:])
        nc.sync.dma_start(bk_sb[:], bk_d[:])
        ones_col = big.tile([1, DK], f32r)
        # DVE Memset can't emit f32r; write the 1.0f bit pattern via u32 view
        nc.vector.memset(ones_col[:].bitcast(u32), 0x3F800000)

        qt = big.tile([128, MT, S], f32r)
        kt_ = big.tile([128, MT, S], f32r)
        vaug = big.tile([128, ST, LH, DK + 1], f32r)
        # flat memset (strided 4D memset is invalid ISA); V-proj drains
        # overwrite cols 0:DK per head, leaving the aug ones-column intact
        nc.vector.memset(vaug[:, :, :, :].bitcast(u32), 0x3F800000)

        def proj_qk(x_dram, w_dram, bias_sb, dst, wname):
            # dst[p, mt, s] = sum_d w[d, mt*128+p] * x^T[d, s] + bias
            # mt pairs share one 2-bank PSUM tile (left/right halves)
            w_sb = wpool.tile([128, KT, HK], f32r, tag="w", name=f"w_{wname}")
            nc.sync.dma_start(w_sb[:], w_dram.ap().rearrange("(kt p) m -> p kt m", p=128))
            for j in range(NB):
                pp = [psS.tile([128, 2 * BLK], f32, tag="psS", name=f"pp_{wname}{j}_{t}")
                      for t in range(2)]
                for kt in range(KT):
                    xt = xs.tile([128, BLK], f32r, tag="xs", name=f"xt_{wname}{j}_{kt}")
                    nc.sync.dma_start(
                        xt[:], x_dram[kt * 128 : (kt + 1) * 128, j * BLK : (j + 1) * BLK]
                    )
                    for mt in range(MT):
                        half = (mt % 2) * BLK
                        nc.tensor.matmul(
                            pp[mt // 2][:, half : half + BLK],
                            w_sb[:, kt, mt * 128 : (mt + 1) * 128],
                            xt[:],
                            start=(kt == 0),
                            stop=(kt == KT - 1),
                            skip_group_check=True,
                        )
                for mt in range(MT):
                    half = (mt % 2) * BLK
                    nc.vector.tensor_scalar_add(
                        dst[:, mt, j * BLK : (j + 1) * BLK],
                        pp[mt // 2][:, half : half + BLK],
                        bias_sb[:, mt : mt + 1],
                    )

        proj_qk(xk_d, wk_d, bk_sb, kt_, "k")
        proj_qk(xq_d, wq_d, bq_sb, qt, "q")

        # V projection: V[st*128+p, h*64+k] per Skv tile st, drained into vaug
        wv_sb = wpool.tile([128, KT, HK], f32r, tag="w")
        nc.sync.dma_start(wv_sb[:], wv_d.ap().rearrange("(kt p) m -> p kt m", p=128))
        for j in range(NB):
            pp = [psS.tile([128, 2 * BLK], f32, tag="psS", name=f"pp_v{j}_{t}")
                  for t in range(2)]
            for kt in range(KT):
                xt = xs.tile([128, BLK], f32r, tag="xs", name=f"xt_v{j}_{kt}")
                nc.sync.dma_start(
                    xt[:], xv_d[kt * 128 : (kt + 1) * 128, j * BLK : (j + 1) * BLK]
                )
                for q in range(4):
                    half = (q % 2) * BLK
                    nc.tensor.matmul(
                        pp[q // 2][:, half : half + BLK],
                        xt[:, q * 128 : (q + 1) * 128],
                        wv_sb[:, kt, :],
                        start=(kt == 0),
                        stop=(kt == KT - 1),
                        skip_group_check=True,
                    )
            for q in range(4):
                st = j * 4 + q
                half = (q % 2) * BLK
                nc.vector.tensor_copy(
                    vaug[:, st, :, 0:DK],
                    pp[q // 2][:, half : half + BLK].rearrange("p (h k) -> p h k", h=LH),
                )

        # Attention + output projection, fused per Sq-block
        wo_sb = wpool.tile([128, MT, D], f32r, tag="w")  # same slot bytes as proj weights
        nc.sync.dma_start(
            wo_sb[:], wo_d.ap().rearrange("(kt p) m -> p kt m", p=128)
        )
        for j in range(NB):
            otj = ot_pool.tile([128, MT, BLK], f32r)
            for hp in range(LH // 2):
                mt = hp
                psu = [psU.tile([DK + 1, BLK], f32, tag="psU",
                                name=f"ps_u{j}_{hp}_{p2}") for p2 in range(2)]
                for i in range(ST):
                    ps2 = psS.tile([128, 2 * BLK], f32, tag="psS",
                                   name=f"ps_s{j}_{hp}_{i}")
                    for pi in range(2):
                        bp = pi * 64
                        nc.tensor.matmul(
                            ps2[:, pi * BLK : (pi + 1) * BLK],
                            kt_[bp : bp + 64, mt, i * 128 : (i + 1) * 128],
                            qt[bp : bp + 64, mt, j * BLK : (j + 1) * BLK],
                            start=True,
                            stop=True,
                            skip_group_check=True,
                        )
                    es = es_pool.tile([128, 2 * BLK], f32r, tag="es")
                    nc.scalar.activation(es[:], ps2[:], Exp)
                    for pi in range(2):
                        h = 2 * hp + pi
                        nc.tensor.matmul(
                            psu[pi][:],
                            vaug[:, i, h, :],
                            es[:, pi * BLK : (pi + 1) * BLK],
                            start=(i == 0),
                            stop=(i == ST - 1),
                            skip_group_check=True,
                        )
                for pi in range(2):
                    bp = pi * 64
                    # r row (psum partition 64) -> partition 0; PE outer-product
                    # broadcasts r to [64, BLK]; then approx-reciprocal aligned
                    # at partition 0 (the custom DVE op ignores input partition
                    # offsets, so it must read from partition 0).
                    rrow = rpool.tile([1, BLK], f32r, tag="r", name=f"rr{j}_{hp}_{pi}")
                    nc.vector.tensor_copy(rrow[:], psu[pi][DK : DK + 1, :])
                    psb = psB.tile([DK, BLK], f32, tag="psB", name=f"psb{j}_{hp}_{pi}")
                    nc.tensor.matmul(
                        psb[:], ones_col[:], rrow[:],
                        start=True, stop=True, skip_group_check=True,
                    )
                    rbc = upool.tile([DK, BLK], f32, tag="rb", name=f"rb{j}_{hp}_{pi}")
                    nc.vector.reciprocal_approx_fast(rbc[:], psb[:])
                    usb = upool.tile([DK, BLK], f32, tag="u", name=f"usb{j}_{hp}_{pi}")
                    nc.vector.tensor_copy(usb[:], psu[pi][0:DK, :])
                    nc.vector.tensor_mul(otj[bp : bp + 64, mt, :], usb[:], rbc[:])
            # output projection for this Sq-block. PSUM from the psU pool so
            # the psS (scores) pool stays free for the next block's scores.
            for mo in range(KT):
                psy = psU.tile([128, BLK], f32, tag="psU", name=f"psy{j}_{mo}")
                for kt in range(MT):
                    nc.tensor.matmul(
                        psy[:],
                        wo_sb[:, kt, mo * 128 : (mo + 1) * 128],
                        otj[:, kt, :],
                        start=(kt == 0),
                        stop=(kt == MT - 1),
                        skip_group_check=True,
                    )
                ysb = ypool.tile([128, BLK], f32, tag="y", name=f"ysb{j}_{mo}")
                nc.vector.tensor_copy(ysb[:], psy[:])
                nc.sync.dma_start(
                    y_d[mo * 128 : (mo + 1) * 128, j * BLK : (j + 1) * BLK], ysb[:]
                )

    nc.compile()
    return nc


def get_program():
    if "nc" not in _CACHE:
        _CACHE["nc"] = _build_program()
    return _CACHE["nc"]


def make_core_inputs(query, key, value, Wq, bq, Wk, bk, Wv, bv, Wo, bo):
    """Build the 8 per-core input dicts (and the folded output bias)."""
    f = np.float32
    in_maps = []
    for c in range(8):
        b, g = c // 2, c % 2
        hs = slice(g * LH, (g + 1) * LH)
        m = {
            "xq_t": np.ascontiguousarray(query[b].T, dtype=f),
            "xk_t": np.ascontiguousarray(key[b].T, dtype=f),
            "xv_t": np.ascontiguousarray(value[b].T, dtype=f),
            "wq": np.ascontiguousarray(
                Wq[hs].transpose(1, 0, 2).reshape(D, HK) / 8.0, dtype=f
            ),
            "wk": np.ascontiguousarray(
                Wk[hs].transpose(1, 0, 2).reshape(D, HK), dtype=f
            ),
            "wv": np.ascontiguousarray(
                Wv[hs].transpose(1, 0, 2).reshape(D, HK), dtype=f
            ),
            "wo": np.ascontiguousarray(Wo[g * HK : (g + 1) * HK, :], dtype=f),
            "bq2": np.ascontiguousarray(
                (bq[hs].reshape(HK) / 8.0).reshape(MT, 128).T, dtype=f
            ),
            "bk2": np.ascontiguousarray(
                bk[hs].reshape(HK).reshape(MT, 128).T, dtype=f
            ),
        }
        in_maps.append(m)
    bo_eff = (bv.reshape(H * DK).astype(np.float64) @ Wo.astype(np.float64)
              + bo.astype(np.float64)).astype(f)
    return in_maps, bo_eff


def combine_outputs(results, bo_eff):
    """results: list of 8 dicts with 'y_t' [D, S]. Returns [B, S, D] f32."""
    out = np.empty((B, S, D), dtype=np.float32)
    for b in range(B):
        acc = results[2 * b]["y_t"] + results[2 * b + 1]["y_t"]
        out[b] = acc.T + bo_eff[None, :]
    return out


def kernel(**inputs):
    from concourse.bass_utils import run_bass_kernel_spmd

    inputs = {k: np.asarray(v) for k, v in inputs.items()}
    nc = get_program()
    in_maps, bo_eff = make_core_inputs(
        inputs["query"], inputs["key"], inputs["value"],
        inputs["Wq"], inputs["bq"], inputs["Wk"], inputs["bk"],
        inputs["Wv"], inputs["bv"], inputs["Wo"], inputs["bo"],
    )
    res = run_bass_kernel_spmd(nc, in_maps, list(range(8)))
    return combine_outputs(res.results, bo_eff)


# revision 10
# speedup vs baseline: 1.9013x; 1.0202x over previous
"""Multi-head attention (B=4, S=2048, D=1024, H=16, dk=64) on 8 TRN2 NeuronCores.

Sharding: core c = (batch b = c//2, head-group g = c%2 of 8 heads).
Each core computes its head-group's attention output and the partial output
projection (Wo rows for its heads); the host sums the two partials per batch
and adds the (folded) output bias.

Per-core math, all in fp32r matmuls (fp32 storage, full-rate reduced-precision
multiply; PSUM accumulation fp32):
  QT = (Wq/8)^T X_q^T + bq/8      [512, 2048]  (heads stacked on partitions)
  KT = Wk^T X_k^T + bk            [512, 2048]
  V  = X_v Wv                     [2048, 512]  + ones column per head (aug)
  per head-pair hp = (h0, h1), Sq-block j:
    S^T_i pair packed in PE row groups 0/64 -> one 2-bank PSUM [128, 1024]
    E_i = exp(S^T pair)           one ScalarE op over 1024 (no max-subtraction
                                   needed: scores ~ N(0,1), max < ~6)
    [U^T; r] += [V_i | 1]^T E_i   [65, 512] per head (row 64 = denominators)
    O^T = U^T * (1/r)             (DVE reciprocal_approx_fast + PE outer bcast)
  Y^T_partial = Wo_c^T O^T        [1024, 2048]
V-bias and output bias are folded on the host: softmax rows sum to 1, so
bv contributes bv_cat @ Wo + bo to every row.

Perf notes (HW-measured):
  - f32r matmul N=512 runs at 227 ns (full rate) incl. overlapped LDWEIGHTS.
  - K=64 matmuls on a single row group run at half rate; alternating row
    groups (base partitions 0/64) makes the pair run concurrently.
  - A [128,1024] 2-bank PSUM tile lets one EXP drain a whole score pair.
"""

import numpy as np

B, S, D = 4, 2048, 1024
H, DK = 16, 64
LH = 8                 # heads per core
HK = LH * DK           # 512 (local concat dim)
BLK = 512              # Sq block size
NB = S // BLK          # 4
ST = S // 128          # 16 Skv tiles
KT = D // 128          # 8 contraction tiles over D
MT = HK // 128         # 4 m-tiles over local heads

_CACHE = {}


def _build_program():
    from contextlib import ExitStack
    import concourse.bass as bass
    import concourse.tile as tile
    from concourse import bacc, mybir

    f32 = mybir.dt.float32
    f32r = mybir.dt.float32r
    bf16 = mybir.dt.bfloat16
    u16 = mybir.dt.uint16
    u32 = mybir.dt.uint32
    Exp = mybir.ActivationFunctionType.Exp

    nc = bacc.Bacc("TRN2", target_bir_lowering=False, debug=False, num_devices=8)

    xq_d = nc.dram_tensor("xq_t", [D, S], f32r, kind="ExternalInput")
    xk_d = nc.dram_tensor("xk_t", [D, S], f32r, kind="ExternalInput")
    xv_d = nc.dram_tensor("xv_t", [D, S], f32r, kind="ExternalInput")
    wq_d = nc.dram_tensor("wq", [D, HK], f32r, kind="ExternalInput")
    wk_d = nc.dram_tensor("wk", [D, HK], f32r, kind="ExternalInput")
    wv_d = nc.dram_tensor("wv", [D, HK], f32r, kind="ExternalInput")
    wo_d = nc.dram_tensor("wo", [HK, D], f32r, kind="ExternalInput")
    bq_d = nc.dram_tensor("bq2", [128, MT], f32, kind="ExternalInput")
    bk_d = nc.dram_tensor("bk2", [128, MT], f32, kind="ExternalInput")
    y_d = nc.dram_tensor("y_t", [D, S], f32, kind="ExternalOutput")

    with tile.TileContext(nc) as tc, ExitStack() as ctx:
        wpool = ctx.enter_context(tc.tile_pool(name="w", bufs=2))
        big = ctx.enter_context(tc.tile_pool(name="big", bufs=1))
        xs = ctx.enter_context(tc.tile_pool(name="xs", bufs=6))
        es_pool = ctx.enter_context(tc.tile_pool(name="es", bufs=8))
        ot_pool = ctx.enter_context(tc.tile_pool(name="ot", bufs=2))
        rpool = ctx.enter_context(tc.tile_pool(name="r", bufs=3))
        upool = ctx.enter_context(tc.tile_pool(name="u", bufs=3))
        ypool = ctx.enter_context(tc.tile_pool(name="y", bufs=3))
        # PSUM: psS 2x[128,1024] (4 banks) + psU 4x[65,512] = 8
        psS = ctx.enter_context(tc.tile_pool(name="psS", bufs=2, space="PSUM"))
        psU = ctx.enter_context(tc.tile_pool(name="psU", bufs=4, space="PSUM"))

        bq_sb = big.tile([128, MT], f32)
        bk_sb = big.tile([128, MT], f32)
        nc.sync.dma_start(bq_sb[:], bq_d[:])
        nc.sync.dma_start(bk_sb[:], bk_d[:])
        qt = big.tile([128, MT, S], f32r)
        kt_ = big.tile([128, MT, S], f32r)
        # V and exp(S) run in bf16: the attention-weight x V product tolerates
        # bf16 (measured 2.2e-3 end-to-end) and bf16 weight loads overlap
        # matmuls (FWL + background weight buffer), unlike f32r ones.
        vaug = big.tile([128, ST, LH, DK + 1], bf16)
        # flat memset (strided 4D memset is invalid ISA); V-proj drains
        # overwrite cols 0:DK per head, leaving the aug ones-column intact
        nc.vector.memset(vaug[:, :, :, :].bitcast(u16), 0x3F80)

        def proj_qk(x_dram, w_dram, bias_sb, dst, wname):
            # dst[p, mt, s] = sum_d w[d, mt*128+p] * x^T[d, s] + bias
            # mt pairs share one 2-bank PSUM tile (left/right halves)
            w_sb = wpool.tile([128, KT, HK], f32r, tag="w", name=f"w_{wname}")
            nc.sync.dma_start(w_sb[:], w_dram.ap().rearrange("(kt p) m -> p kt m", p=128))
            for j in range(NB):
                pp = [psS.tile([128, 2 * BLK], f32, tag="psS", name=f"pp_{wname}{j}_{t}")
                      for t in range(2)]
                for kt in range(KT):
                    xt = xs.tile([128, BLK], f32r, tag="xs", name=f"xt_{wname}{j}_{kt}")
                    nc.sync.dma_start(
                        xt[:], x_dram[kt * 128 : (kt + 1) * 128, j * BLK : (j + 1) * BLK]
                    )
                    for mt in range(MT):
                        half = (mt % 2) * BLK
                        nc.tensor.matmul(
                            pp[mt // 2][:, half : half + BLK],
                            w_sb[:, kt, mt * 128 : (mt + 1) * 128],
                            xt[:],
                            start=(kt == 0),
                            stop=(kt == KT - 1),
                            skip_group_check=True,
                        )
                for mt in range(MT):
                    half = (mt % 2) * BLK
                    nc.vector.tensor_scalar_add(
                        dst[:, mt, j * BLK : (j + 1) * BLK],
                        pp[mt // 2][:, half : half + BLK],
                        bias_sb[:, mt : mt + 1],
                    )

        proj_qk(xk_d, wk_d, bk_sb, kt_, "k")
        proj_qk(xq_d, wq_d, bq_sb, qt, "q")

        # V projection: V[st*128+p, h*64+k] per Skv tile st, drained into vaug
        wv_sb = wpool.tile([128, KT, HK], f32r, tag="w")
        nc.sync.dma_start(wv_sb[:], wv_d.ap().rearrange("(kt p) m -> p kt m", p=128))
        for j in range(NB):
            pp = [psS.tile([128, 2 * BLK], f32, tag="psS", name=f"pp_v{j}_{t}")
                  for t in range(2)]
            for kt in range(KT):
                xt = xs.tile([128, BLK], f32r, tag="xs", name=f"xt_v{j}_{kt}")
                nc.sync.dma_start(
                    xt[:], xv_d[kt * 128 : (kt + 1) * 128, j * BLK : (j + 1) * BLK]
                )
                for q in range(4):
                    half = (q % 2) * BLK
                    nc.tensor.matmul(
                        pp[q // 2][:, half : half + BLK],
                        xt[:, q * 128 : (q + 1) * 128],
                        wv_sb[:, kt, :],
                        start=(kt == 0),
                        stop=(kt == KT - 1),
                        skip_group_check=True,
                    )
            for q in range(4):
                st = j * 4 + q
                half = (q % 2) * BLK
                nc.vector.tensor_copy(
                    vaug[:, st, :, 0:DK],
                    pp[q // 2][:, half : half + BLK].rearrange("p (h k) -> p h k", h=LH),
                )

        # Attention + output projection, fused per Sq-block
        wo_sb = wpool.tile([128, MT, D], f32r, tag="w")  # same slot bytes as proj weights
        nc.sync.dma_start(
            wo_sb[:], wo_d.ap().rearrange("(kt p) m -> p kt m", p=128)
        )
        for j in range(NB):
            otj = ot_pool.tile([128, MT, BLK], f32r)
            for hp in range(LH // 2):
                mt = hp
                psu = [psU.tile([DK + 1, BLK], f32, tag="psU",
                                name=f"ps_u{j}_{hp}_{p2}") for p2 in range(2)]
                for i in range(ST):
                    ps2 = psS.tile([128, 2 * BLK], f32, tag="psS",
                                   name=f"ps_s{j}_{hp}_{i}")
                    for pi in range(2):
                        bp = pi * 64
                        nc.tensor.matmul(
                            ps2[:, pi * BLK : (pi + 1) * BLK],
                            kt_[bp : bp + 64, mt, i * 128 : (i + 1) * 128],
                            qt[bp : bp + 64, mt, j * BLK : (j + 1) * BLK],
                            start=True,
                            stop=True,
                            skip_group_check=True,
                        )
                    es = es_pool.tile([128, 2 * BLK], bf16, tag="es")
                    nc.scalar.activation(es[:], ps2[:], Exp)
                    for pi in range(2):
                        h = 2 * hp + pi
                        nc.tensor.matmul(
                            psu[pi][:],
                            vaug[:, i, h, :],
                            es[:, pi * BLK : (pi + 1) * BLK],
                            start=(i == 0),
                            stop=(i == ST - 1),
                            skip_group_check=True,
                        )
                for pi in range(2):
                    bp = pi * 64
                    # r row (psum partition 64) -> partition 0 (plain DVE copy
                    # handles the shift; the custom reciprocal op does not, so
                    # it runs after, aligned at partition 0), then broadcast
                    # 1/r across 64 partitions on the idle GpSimd engine.
                    rrow = rpool.tile([1, BLK], f32, tag="r", name=f"rr{j}_{hp}_{pi}")
                    nc.vector.tensor_copy(rrow[:], psu[pi][DK : DK + 1, :])
                    rf = rpool.tile([1, BLK], f32, tag="rf", name=f"rf{j}_{hp}_{pi}")
                    nc.vector.reciprocal_approx_fast(rf[:], rrow[:])
                    rbc = upool.tile([DK, BLK], f32, tag="rb", name=f"rb{j}_{hp}_{pi}")
                    nc.gpsimd.partition_broadcast(rbc[:], rf[:])
                    nc.vector.tensor_mul(otj[bp : bp + 64, mt, :],
                                         psu[pi][0:DK, :], rbc[:])
            # output projection for this Sq-block. PSUM from the psU pool so
            # the psS (scores) pool stays free for the next block's scores.
            for mo in range(KT):
                psy = psU.tile([128, BLK], f32, tag="psU", name=f"psy{j}_{mo}")
                for kt in range(MT):
                    nc.tensor.matmul(
                        psy[:],
                        wo_sb[:, kt, mo * 128 : (mo + 1) * 128],
                        otj[:, kt, :],
                        start=(kt == 0),
                        stop=(kt == MT - 1),
                        skip_group_check=True,
                    )
                ysb = ypool.tile([128, BLK], f32, tag="y", name=f"ysb{j}_{mo}")
                nc.vector.tensor_copy(ysb[:], psy[:])
                nc.sync.dma_start(
                    y_d[mo * 128 : (mo + 1) * 128, j * BLK : (j + 1) * BLK], ysb[:]
                )

    nc.compile()
    return nc


def get_program():
    if "nc" not in _CACHE:
        _CACHE["nc"] = _build_program()
    return _CACHE["nc"]


def make_core_inputs(query, key, value, Wq, bq, Wk, bk, Wv, bv, Wo, bo):
    """Build the 8 per-core input dicts (and the folded output bias)."""
    f = np.float32
    in_maps = []
    for c in range(8):
        b, g = c // 2, c % 2
        hs = slice(g * LH, (g + 1) * LH)
        m = {
            "xq_t": np.ascontiguousarray(query[b].T, dtype=f),
            "xk_t": np.ascontiguousarray(key[b].T, dtype=f),
            "xv_t": np.ascontiguousarray(value[b].T, dtype=f),
            "wq": np.ascontiguousarray(
                Wq[hs].transpose(1, 0, 2).reshape(D, HK) / 8.0, dtype=f
            ),
            "wk": np.ascontiguousarray(
                Wk[hs].transpose(1, 0, 2).reshape(D, HK), dtype=f
            ),
            "wv": np.ascontiguousarray(
                Wv[hs].transpose(1, 0, 2).reshape(D, HK), dtype=f
            ),
            "wo": np.ascontiguousarray(Wo[g * HK : (g + 1) * HK, :], dtype=f),
            "bq2": np.ascontiguousarray(
                (bq[hs].reshape(HK) / 8.0).reshape(MT, 128).T, dtype=f
            ),
            "bk2": np.ascontiguousarray(
                bk[hs].reshape(HK).reshape(MT, 128).T, dtype=f
            ),
        }
        in_maps.append(m)
    bo_eff = (bv.reshape(H * DK).astype(np.float64) @ Wo.astype(np.float64)
              + bo.astype(np.float64)).astype(f)
    return in_maps, bo_eff


def combine_outputs(results, bo_eff):
    """results: list of 8 dicts with 'y_t' [D, S]. Returns [B, S, D] f32."""
    out = np.empty((B, S, D), dtype=np.float32)
    for b in range(B):
        acc = results[2 * b]["y_t"] + results[2 * b + 1]["y_t"]
        out[b] = acc.T + bo_eff[None, :]
    return out


def kernel(**inputs):
    from concourse.bass_utils import run_bass_kernel_spmd

    inputs = {k: np.asarray(v) for k, v in inputs.items()}
    nc = get_program()
    in_maps, bo_eff = make_core_inputs(
        inputs["query"], inputs["key"], inputs["value"],
        inputs["Wq"], inputs["bq"], inputs["Wk"], inputs["bk"],
        inputs["Wv"], inputs["bv"], inputs["Wo"], inputs["bo"],
    )
    res = run_bass_kernel_spmd(nc, in_maps, list(range(8)))
    return combine_outputs(res.results, bo_eff)
